# revision 6
# baseline (speedup 1.0000x reference)
"""Trainium2 Bass kernel for nn_MessagePassing (gnn_message_passing).

Math (per batch b):
    coef[s,e] = sum_o adj[s,o] * edge[s,o,e]
    v[s,e,i]  = sum_j W[e,i,j] * node[s,j]
    out[s,i]  = sum_e coef[s,e] * v[s,e,i]

Sharding: data parallel over the batch axis - core b handles batch b.

Host-side staging (per core):
  * edge  -> [s, e, o] bf16: contiguous on-chip reads, half the HBM bytes.
  * adj   -> [p, t, o] bf16, node -> nodeT [j, s] bf16, W -> wT [j, e, i]
    bf16, sel -> e-row selector matrices (constant).
  * out   <- [p, t, i] f32, one contiguous DMA.

Engine split per s-tile (measured HW rates, ns per 1024 elem/partition):
DVE STT fused mult+reduce ~1226, DVE TT bf16-packed-2x ~553 (works with a
stride-0 middle-dim broadcast), DVE TRED ~1086, ACT copy+accum ~1147+278
drain (concurrent with DVE), Pool shares the DVE SBUF port (useless).

  * coef: a_t e's (2 or 3, alternating to balance DVE vs ACT) via fused
    DVE STT; the rest via one 2x DVE TT multiply (adj broadcast over the
    middle dim) + ACT copy+accum reductions.
  * out = sum_e coef[s,e] * (node[s,:] @ W_e) is computed entirely on the
    PE as 8 PSUM-accumulated matmuls with lhsT_e = uT[j,e,s] =
    nodeT[j,s]*coef[s,e].  coef[s,e] must be replicated across the j
    partitions: PE-transpose coef -> coefT[e,s], then 8 selector matmuls
    (lhsT = const one-hot row e) broadcast coefT rows into PSUM, and one
    2x-ineligible DVE TT (PSUM operand) forms uT.
"""

import numpy as np
import ml_dtypes
from contextlib import ExitStack

import concourse.bass as bass
import concourse.bacc as bacc
import concourse.mybir as mybir
import concourse.tile as tile
from concourse.bass_utils import run_bass_kernel_spmd
from concourse.masks import make_identity

B, N, D, E = 8, 1024, 128, 8
P = 128
NT = N // P  # 8 s-tiles per core
CH = 3  # edge chunk split: e<CH arrives first (covers all STT e's)

F32 = mybir.dt.float32
BF16 = mybir.dt.bfloat16
MUL = mybir.AluOpType.mult
ADD = mybir.AluOpType.add
COPY = mybir.ActivationFunctionType.Copy

BF16_NP = ml_dtypes.bfloat16


def build_nc():
    nc = bacc.Bacc("TRN2", target_bir_lowering=False, debug=False, num_devices=B)

    edge_d = nc.dram_tensor("edge_t", [N, E, N], BF16, kind="ExternalInput").ap()
    adj_d = nc.dram_tensor("adj_r", [P, NT, N], BF16, kind="ExternalInput").ap()
    nodeT_d = nc.dram_tensor("nodeT", [D, N], BF16, kind="ExternalInput").ap()
    wT_d = nc.dram_tensor("wT", [D, E, D], BF16, kind="ExternalInput").ap()
    sel_d = nc.dram_tensor("sel", [E, E, P], BF16, kind="ExternalInput").ap()
    out_d = nc.dram_tensor("out", [P, NT, D], F32, kind="ExternalOutput").ap()

    with tile.TileContext(nc) as tc, ExitStack() as ctx:
        const_pool = ctx.enter_context(tc.tile_pool(name="const", bufs=1))
        edge_pool = ctx.enter_context(tc.tile_pool(name="edge", bufs=4))
        prod_pool = ctx.enter_context(tc.tile_pool(name="prod", bufs=3))
        work_pool = ctx.enter_context(tc.tile_pool(name="work", bufs=3))
        psum_pool = ctx.enter_context(tc.tile_pool(name="psum", bufs=2, space="PSUM"))

        adj_all = const_pool.tile([P, NT, N], BF16)
        nodeT = const_pool.tile([P, N], BF16)
        wT = const_pool.tile([P, E, D], BF16)
        sel = const_pool.tile([E, E, P], BF16)
        ident = const_pool.tile([P, P], BF16)
        acc_all = const_pool.tile([P, NT, D], F32)
        scr_v = const_pool.tile([P, N], BF16)  # DVE STT product sink
        scr_a = const_pool.tile([P, N], BF16)  # ACT copy sink

        make_identity(nc, ident[:])

        # Edge stream on the sync queue (tile 0 split so the first STT can
        # start after ~0.8 MiB); everything else on the scalar queue.
        def load_edge(t, split=False):
            et = edge_pool.tile([P, E, N], BF16, tag="edge_t")
            if split:
                nc.sync.dma_start(et[:, 0:CH, :], edge_d[bass.ts(t, P), 0:CH])
                nc.sync.dma_start(et[:, CH:E, :], edge_d[bass.ts(t, P), CH:E])
            else:
                nc.sync.dma_start(et[:], edge_d[bass.ts(t, P)])
            return et

        edge_tiles = {0: load_edge(0, split=True)}
        nc.scalar.dma_start(adj_all[:, 0, :], adj_d[:, 0, :])
        nc.scalar.dma_start(nodeT[:], nodeT_d)
        nc.scalar.dma_start(wT[:], wT_d)
        nc.scalar.dma_start(sel[:], sel_d)
        for t in range(1, NT):
            edge_tiles[t] = load_edge(t)
            nc.scalar.dma_start(adj_all[:, t, :], adj_d[:, t, :])

        A_PAT = [3, 2, 2, 3, 2, 2, 3, 2]  # STT e's per tile (avg 2.375)

        def coef_stage(t):
            edge_t = edge_tiles[t]
            adj_t = adj_all[:, t, :]
            a = A_PAT[t]
            coef = work_pool.tile([P, E], F32, tag="coef")
            # DVE: fused multiply+reduce for e < a
            for e in range(a):
                nc.vector.scalar_tensor_tensor(
                    out=scr_v[:],
                    in0=edge_t[:, e, :],
                    scalar=1.0,
                    in1=adj_t,
                    op0=MUL,
                    op1=MUL,
                    accum_out=coef[:, e : e + 1],
                )
            # DVE: 2x TT multiply for e >= a (adj broadcast over middle dim)
            prod = prod_pool.tile([P, E - 2, N], BF16, tag="prod")
            nc.vector.tensor_tensor(
                out=prod[:, 0 : E - a, :],
                in0=edge_t[:, a:E, :],
                in1=adj_t[:, None, :].broadcast_to((P, E - a, N)),
                op=MUL,
            )
            # ACT: reduce each prod slice -> coef[:, e]
            for e in range(a, E):
                nc.scalar.activation(
                    out=scr_a[:],
                    in_=prod[:, e - a, :],
                    func=COPY,
                    accum_out=coef[:, e : e + 1],
                )
            return coef

        def out_stage(t, coef):
            # coef -> bf16, PE-transpose, broadcast across partitions
            coef16 = work_pool.tile([P, E], BF16, tag="coef16")
            nc.vector.tensor_scalar_mul(coef16[:], coef[:], 1.0)
            pT = psum_pool.tile([E, P], BF16, tag="pT")
            nc.tensor.transpose(pT[:], coef16[:], ident[:])
            coefT = work_pool.tile([E, P], BF16, tag="coefT")
            nc.vector.tensor_scalar_mul(coefT[:], pT[:], 1.0)
            coefB = psum_pool.tile([P, E, P], F32, tag="cB")
            for e in range(E):
                nc.tensor.matmul(
                    coefB[:, e, :], lhsT=sel[:, e, :], rhs=coefT[:],
                    start=True, stop=True,
                )
            # uT[j, e, s] = nodeT[j, s] * coef[s, e]
            uT = work_pool.tile([P, E, P], BF16, tag="uT")
            nc.vector.tensor_tensor(
                out=uT[:],
                in0=nodeT[:, bass.ts(t, P)][:, None, :].broadcast_to((P, E, P)),
                in1=coefB[:],
                op=MUL,
            )
            # out[s, i] = sum_e uT_e^T @ W_e, accumulated in PSUM
            po = psum_pool.tile([P, D], F32, tag="po")
            for e in range(E):
                nc.tensor.matmul(
                    po[:], lhsT=uT[:, e, :], rhs=wT[:, e, :],
                    start=(e == 0), stop=(e == E - 1),
                )
            nc.vector.tensor_scalar_mul(acc_all[:, t, :], po[:], 1.0)

        # Software pipeline: emit tile t+1's coef work before tile t's
        # out-chain so the DVE streams while the out-chain latency drains.
        coefs = {0: coef_stage(0)}
        for t in range(NT):
            if t + 1 < NT:
                coefs[t + 1] = coef_stage(t + 1)
            out_stage(t, coefs.pop(t))

        nc.gpsimd.dma_start(out_d, acc_all[:])

    nc.compile()
    return nc


_NC_CACHE = None


def get_nc():
    global _NC_CACHE
    if _NC_CACHE is None:
        _NC_CACHE = build_nc()
    return _NC_CACHE


def make_in_maps(node_state, edge_type_mat, adj_mat, W):
    node_state = np.asarray(node_state, dtype=np.float32)
    edge_type_mat = np.asarray(edge_type_mat, dtype=np.float32)
    adj_mat = np.asarray(adj_mat, dtype=np.float32)
    W = np.asarray(W, dtype=np.float32)

    wT = np.ascontiguousarray(W.transpose(2, 0, 1)).astype(BF16_NP)  # [j, e, i]
    sel = np.zeros((E, E, P), dtype=np.float32)
    for e in range(E):
        sel[e, e, :] = 1.0
    sel = sel.astype(BF16_NP)
    in_maps = []
    for b in range(B):
        edge16 = edge_type_mat[b].astype(BF16_NP)  # [s, o, e]
        edge_t = np.ascontiguousarray(edge16.transpose(0, 2, 1))  # [s, e, o]
        adj16 = adj_mat[b].astype(BF16_NP).reshape(NT, P, N)
        adj_r = np.ascontiguousarray(adj16.transpose(1, 0, 2))  # [p, t, o]
        nodeT = np.ascontiguousarray(node_state[b].T).astype(BF16_NP)  # [j, s]
        in_maps.append(
            {"edge_t": edge_t, "adj_r": adj_r, "nodeT": nodeT, "wT": wT, "sel": sel}
        )
    return in_maps


def kernel(node_state, edge_type_mat, adj_mat, W):
    nc = get_nc()
    in_maps = make_in_maps(node_state, edge_type_mat, adj_mat, W)
    res = run_bass_kernel_spmd(nc, in_maps, list(range(B)))
    # out is [p, t, i] per core -> [s, i] with s = t*P + p
    return np.stack(
        [res.results[b]["out"].transpose(1, 0, 2).reshape(N, D) for b in range(B)],
        axis=0,
    )


# revision 7
# speedup vs baseline: 1.0093x; 1.0093x over previous
"""Trainium2 Bass kernel for nn_MessagePassing (gnn_message_passing).

Math (per batch b):
    coef[s,e] = sum_o adj[s,o] * edge[s,o,e]
    v[s,e,i]  = sum_j W[e,i,j] * node[s,j]
    out[s,i]  = sum_e coef[s,e] * v[s,e,i]

Sharding: data parallel over the batch axis - core b handles batch b.

Host-side staging (per core):
  * edge  -> [s, e, o] bf16: contiguous on-chip reads, half the HBM bytes.
  * adj   -> [p, t, o] bf16, node -> nodeT [j, s] bf16, W -> wT [j, e, i]
    bf16, sel -> e-row selector matrices (constant).
  * out   <- [p, t, i] f32, one contiguous DMA.

Engine split per s-tile (measured HW rates, ns per 1024 elem/partition):
DVE STT fused mult+reduce ~1226, DVE TT bf16-packed-2x ~553 (works with a
stride-0 middle-dim broadcast), DVE TRED ~1086, ACT copy+accum ~1147+278
drain (concurrent with DVE), Pool shares the DVE SBUF port (useless).

  * coef: a_t e's (2 or 3, alternating to balance DVE vs ACT) via fused
    DVE STT; the rest via one 2x DVE TT multiply (adj broadcast over the
    middle dim) + ACT copy+accum reductions.
  * out = sum_e coef[s,e] * (node[s,:] @ W_e) is computed entirely on the
    PE as 8 PSUM-accumulated matmuls with lhsT_e = uT[j,e,s] =
    nodeT[j,s]*coef[s,e].  coef[s,e] must be replicated across the j
    partitions: PE-transpose coef -> coefT[e,s], then 8 selector matmuls
    (lhsT = const one-hot row e) broadcast coefT rows into PSUM, and one
    2x-ineligible DVE TT (PSUM operand) forms uT.
"""

import numpy as np
import ml_dtypes
from contextlib import ExitStack

import concourse.bass as bass
import concourse.bacc as bacc
import concourse.mybir as mybir
import concourse.tile as tile
from concourse.bass_utils import run_bass_kernel_spmd
from concourse.masks import make_identity

B, N, D, E = 8, 1024, 128, 8
P = 128
NT = N // P  # 8 s-tiles per core
CH = 3  # edge chunk split: e<CH arrives first (covers all STT e's)

F32 = mybir.dt.float32
BF16 = mybir.dt.bfloat16
MUL = mybir.AluOpType.mult
ADD = mybir.AluOpType.add
COPY = mybir.ActivationFunctionType.Copy

BF16_NP = ml_dtypes.bfloat16


def build_nc():
    nc = bacc.Bacc("TRN2", target_bir_lowering=False, debug=False, num_devices=B)

    edge_d = nc.dram_tensor("edge_t", [N, E, N], BF16, kind="ExternalInput").ap()
    adj_d = nc.dram_tensor("adj_r", [P, NT, N], BF16, kind="ExternalInput").ap()
    nodeT_d = nc.dram_tensor("nodeT", [D, N], BF16, kind="ExternalInput").ap()
    wT_d = nc.dram_tensor("wT", [D, E, D], BF16, kind="ExternalInput").ap()
    sel_d = nc.dram_tensor("sel", [E, E, P], BF16, kind="ExternalInput").ap()
    out_d = nc.dram_tensor("out", [P, NT, D], F32, kind="ExternalOutput").ap()

    with tile.TileContext(nc) as tc, ExitStack() as ctx:
        const_pool = ctx.enter_context(tc.tile_pool(name="const", bufs=1))
        edge_pool = ctx.enter_context(tc.tile_pool(name="edge", bufs=4))
        prod_pool = ctx.enter_context(tc.tile_pool(name="prod", bufs=3))
        work_pool = ctx.enter_context(tc.tile_pool(name="work", bufs=3))
        psum_pool = ctx.enter_context(tc.tile_pool(name="psum", bufs=2, space="PSUM"))

        adj_all = const_pool.tile([P, NT, N], BF16)
        nodeT = const_pool.tile([P, N], BF16)
        wT = const_pool.tile([P, E, D], BF16)
        sel = const_pool.tile([E, E, P], BF16)
        ident = const_pool.tile([P, P], BF16)
        acc_all = const_pool.tile([P, NT, D], F32)
        scr_v = const_pool.tile([P, N], BF16)  # DVE STT product sink
        scr_a = const_pool.tile([P, N], BF16)  # ACT copy sink

        make_identity(nc, ident[:])

        # Edge stream on the sync queue (tile 0 split so the first STT can
        # start after ~0.8 MiB); everything else on the scalar queue.
        def load_edge(t, split=False):
            et = edge_pool.tile([P, E, N], BF16, tag="edge_t")
            if split:
                nc.sync.dma_start(et[:, 0:CH, :], edge_d[bass.ts(t, P), 0:CH])
                nc.sync.dma_start(et[:, CH:E, :], edge_d[bass.ts(t, P), CH:E])
            else:
                nc.sync.dma_start(et[:], edge_d[bass.ts(t, P)])
            return et

        edge_tiles = {0: load_edge(0, split=True)}
        nc.scalar.dma_start(adj_all[:, 0, :], adj_d[:, 0, :])
        nc.scalar.dma_start(nodeT[:], nodeT_d)
        nc.scalar.dma_start(wT[:], wT_d)
        nc.scalar.dma_start(sel[:], sel_d)
        for t in range(1, NT):
            edge_tiles[t] = load_edge(t)
            nc.scalar.dma_start(adj_all[:, t, :], adj_d[:, t, :])

        A_PAT = [3, 2, 2, 3, 2, 2, 3, 2]  # STT e's per tile (avg 2.375)

        def coef_stage(t):
            edge_t = edge_tiles[t]
            adj_t = adj_all[:, t, :]
            a = A_PAT[t]
            coef = work_pool.tile([P, E], F32, tag="coef")
            # DVE: fused multiply+reduce for e < a
            for e in range(a):
                nc.vector.scalar_tensor_tensor(
                    out=scr_v[:],
                    in0=edge_t[:, e, :],
                    scalar=1.0,
                    in1=adj_t,
                    op0=MUL,
                    op1=MUL,
                    accum_out=coef[:, e : e + 1],
                )
            # DVE: 2x TT multiply for e >= a (adj broadcast over middle dim)
            prod = prod_pool.tile([P, E - 2, N], BF16, tag="prod")
            nc.vector.tensor_tensor(
                out=prod[:, 0 : E - a, :],
                in0=edge_t[:, a:E, :],
                in1=adj_t[:, None, :].broadcast_to((P, E - a, N)),
                op=MUL,
            )
            # ACT: reduce each prod slice -> coef[:, e]
            for e in range(a, E):
                nc.scalar.activation(
                    out=scr_a[:],
                    in_=prod[:, e - a, :],
                    func=COPY,
                    accum_out=coef[:, e : e + 1],
                )
            return coef

        def out_stage(t, coef):
            # coef -> bf16, PE-transpose, broadcast across partitions
            coef16 = work_pool.tile([P, E], BF16, tag="coef16")
            nc.vector.tensor_scalar_mul(coef16[:], coef[:], 1.0)
            pT = psum_pool.tile([E, P], BF16, tag="pT")
            nc.tensor.transpose(pT[:], coef16[:], ident[:])
            coefT = work_pool.tile([E, P], BF16, tag="coefT")
            nc.vector.tensor_scalar_mul(coefT[:], pT[:], 1.0)
            coefB = psum_pool.tile([P, E, P], F32, tag="cB")
            for e in range(E):
                nc.tensor.matmul(
                    coefB[:, e, :], lhsT=sel[:, e, :], rhs=coefT[:],
                    start=True, stop=True,
                )
            # uT[j, e, s] = nodeT[j, s] * coef[s, e]
            uT = work_pool.tile([P, E, P], BF16, tag="uT")
            nc.vector.tensor_tensor(
                out=uT[:],
                in0=nodeT[:, bass.ts(t, P)][:, None, :].broadcast_to((P, E, P)),
                in1=coefB[:],
                op=MUL,
            )
            # out[s, i] = sum_e uT_e^T @ W_e, accumulated in PSUM
            po = psum_pool.tile([P, D], F32, tag="po")
            for e in range(E):
                nc.tensor.matmul(
                    po[:], lhsT=uT[:, e, :], rhs=wT[:, e, :],
                    start=(e == 0), stop=(e == E - 1),
                )
            nc.vector.tensor_scalar_mul(acc_all[:, t, :], po[:], 1.0)

        for t in range(NT):
            out_stage(t, coef_stage(t))

        nc.gpsimd.dma_start(out_d, acc_all[:])

    nc.compile()
    return nc


_NC_CACHE = None


def get_nc():
    global _NC_CACHE
    if _NC_CACHE is None:
        _NC_CACHE = build_nc()
    return _NC_CACHE


def make_in_maps(node_state, edge_type_mat, adj_mat, W):
    node_state = np.asarray(node_state, dtype=np.float32)
    edge_type_mat = np.asarray(edge_type_mat, dtype=np.float32)
    adj_mat = np.asarray(adj_mat, dtype=np.float32)
    W = np.asarray(W, dtype=np.float32)

    wT = np.ascontiguousarray(W.transpose(2, 0, 1)).astype(BF16_NP)  # [j, e, i]
    sel = np.zeros((E, E, P), dtype=np.float32)
    for e in range(E):
        sel[e, e, :] = 1.0
    sel = sel.astype(BF16_NP)
    in_maps = []
    for b in range(B):
        edge16 = edge_type_mat[b].astype(BF16_NP)  # [s, o, e]
        edge_t = np.ascontiguousarray(edge16.transpose(0, 2, 1))  # [s, e, o]
        adj16 = adj_mat[b].astype(BF16_NP).reshape(NT, P, N)
        adj_r = np.ascontiguousarray(adj16.transpose(1, 0, 2))  # [p, t, o]
        nodeT = np.ascontiguousarray(node_state[b].T).astype(BF16_NP)  # [j, s]
        in_maps.append(
            {"edge_t": edge_t, "adj_r": adj_r, "nodeT": nodeT, "wT": wT, "sel": sel}
        )
    return in_maps


def kernel(node_state, edge_type_mat, adj_mat, W):
    nc = get_nc()
    in_maps = make_in_maps(node_state, edge_type_mat, adj_mat, W)
    res = run_bass_kernel_spmd(nc, in_maps, list(range(B)))
    # out is [p, t, i] per core -> [s, i] with s = t*P + p
    return np.stack(
        [res.results[b]["out"].transpose(1, 0, 2).reshape(N, D) for b in range(B)],
        axis=0,
    )


# revision 8
# speedup vs baseline: 1.0283x; 1.0189x over previous
"""Trainium2 Bass kernel for nn_MessagePassing (gnn_message_passing).

Math (per batch b):
    coef[s,e] = sum_o adj[s,o] * edge[s,o,e]
    v[s,e,i]  = sum_j W[e,i,j] * node[s,j]
    out[s,i]  = sum_e coef[s,e] * v[s,e,i]

Sharding: data parallel over the batch axis - core b handles batch b.

Host-side staging (per core):
  * edge  -> [s, e, o] bf16: contiguous on-chip reads, half the HBM bytes.
  * adj   -> [p, t, o] bf16, node -> nodeT [j, s] bf16, W -> wT [j, e, i]
    bf16, sel -> e-row selector matrices (constant).
  * out   <- [p, t, i] f32, one contiguous DMA.

Engine split per s-tile (measured HW rates, ns per 1024 elem/partition):
DVE STT fused mult+reduce ~1226, DVE TT bf16-packed-2x ~553 (works with a
stride-0 middle-dim broadcast), DVE TRED ~1086, ACT copy+accum ~1147+278
drain (concurrent with DVE), Pool shares the DVE SBUF port (useless).

  * coef: a_t e's (2 or 3, alternating to balance DVE vs ACT) via fused
    DVE STT; the rest via one 2x DVE TT multiply (adj broadcast over the
    middle dim) + ACT copy+accum reductions.
  * out = sum_e coef[s,e] * (node[s,:] @ W_e) is computed entirely on the
    PE as 8 PSUM-accumulated matmuls with lhsT_e = uT[j,e,s] =
    nodeT[j,s]*coef[s,e].  coef[s,e] must be replicated across the j
    partitions: PE-transpose coef -> coefT[e,s], then 8 selector matmuls
    (lhsT = const one-hot row e) broadcast coefT rows into PSUM, and one
    2x-ineligible DVE TT (PSUM operand) forms uT.
"""

import numpy as np
import ml_dtypes
from contextlib import ExitStack

import concourse.bass as bass
import concourse.bacc as bacc
import concourse.mybir as mybir
import concourse.tile as tile
from concourse.bass_utils import run_bass_kernel_spmd
from concourse.masks import make_identity

B, N, D, E = 8, 1024, 128, 8
P = 128
NT = N // P  # 8 s-tiles per core
CH = 3  # edge chunk split: e<CH arrives first (covers all STT e's)

F32 = mybir.dt.float32
BF16 = mybir.dt.bfloat16
MUL = mybir.AluOpType.mult
ADD = mybir.AluOpType.add
COPY = mybir.ActivationFunctionType.Copy

BF16_NP = ml_dtypes.bfloat16


def build_nc():
    nc = bacc.Bacc("TRN2", target_bir_lowering=False, debug=False, num_devices=B)

    edge_d = nc.dram_tensor("edge_t", [N, E, N], BF16, kind="ExternalInput").ap()
    adj_d = nc.dram_tensor("adj_r", [P, NT, N], BF16, kind="ExternalInput").ap()
    nodeT_d = nc.dram_tensor("nodeT", [D, N], BF16, kind="ExternalInput").ap()
    wT_d = nc.dram_tensor("wT", [D, E, D], BF16, kind="ExternalInput").ap()
    sel_d = nc.dram_tensor("sel", [E, E, P], BF16, kind="ExternalInput").ap()
    out_d = nc.dram_tensor("out", [P, NT, D], F32, kind="ExternalOutput").ap()

    with tile.TileContext(nc) as tc, ExitStack() as ctx:
        const_pool = ctx.enter_context(tc.tile_pool(name="const", bufs=1))
        edge_pool = ctx.enter_context(tc.tile_pool(name="edge", bufs=3))
        prod_pool = ctx.enter_context(tc.tile_pool(name="prod", bufs=2))
        work_pool = ctx.enter_context(tc.tile_pool(name="work", bufs=2))
        psum_pool = ctx.enter_context(tc.tile_pool(name="psum", bufs=2, space="PSUM"))

        adj_all = const_pool.tile([P, NT, N], BF16)
        nodeT = const_pool.tile([P, N], BF16)
        wT = const_pool.tile([P, E, D], BF16)
        sel = const_pool.tile([E, E, P], BF16)
        ident = const_pool.tile([P, P], BF16)
        acc_all = const_pool.tile([P, NT, D], F32)
        scr_v = const_pool.tile([P, N], BF16)  # DVE STT product sink
        scr_a = const_pool.tile([P, N], BF16)  # ACT copy sink

        make_identity(nc, ident[:])

        # Edge stream on the sync queue (tile 0 split so the first STT can
        # start after ~0.8 MiB); everything else on the scalar queue.
        def load_edge(t, split=False):
            et = edge_pool.tile([P, E, N], BF16, tag="edge_t")
            if split:
                nc.sync.dma_start(et[:, 0:CH, :], edge_d[bass.ts(t, P), 0:CH])
                nc.sync.dma_start(et[:, CH:E, :], edge_d[bass.ts(t, P), CH:E])
            else:
                nc.sync.dma_start(et[:], edge_d[bass.ts(t, P)])
            return et

        edge_tiles = {0: load_edge(0, split=True)}
        nc.scalar.dma_start(adj_all[:, 0, :], adj_d[:, 0, :])
        nc.scalar.dma_start(nodeT[:], nodeT_d)
        nc.scalar.dma_start(wT[:], wT_d)
        nc.scalar.dma_start(sel[:], sel_d)
        for t in range(1, NT):
            edge_tiles[t] = load_edge(t)
            nc.scalar.dma_start(adj_all[:, t, :], adj_d[:, t, :])

        A_PAT = [3, 3, 3, 2, 3, 3, 2, 3]  # STT e's per tile
        for t in range(NT):
            edge_t = edge_tiles[t]
            adj_t = adj_all[:, t, :]
            a = A_PAT[t]

            coef = work_pool.tile([P, E], F32)
            # DVE: fused multiply+reduce for e < a
            for e in range(a):
                nc.vector.scalar_tensor_tensor(
                    out=scr_v[:],
                    in0=edge_t[:, e, :],
                    scalar=1.0,
                    in1=adj_t,
                    op0=MUL,
                    op1=MUL,
                    accum_out=coef[:, e : e + 1],
                )
            # DVE: 2x TT multiply for e >= a (adj broadcast over middle dim)
            prod = prod_pool.tile([P, E - 2, N], BF16)
            nc.vector.tensor_tensor(
                out=prod[:, 0 : E - a, :],
                in0=edge_t[:, a:E, :],
                in1=adj_t[:, None, :].broadcast_to((P, E - a, N)),
                op=MUL,
            )
            # ACT: reduce each prod slice -> coef[:, e]
            for e in range(a, E):
                nc.scalar.activation(
                    out=scr_a[:],
                    in_=prod[:, e - a, :],
                    func=COPY,
                    accum_out=coef[:, e : e + 1],
                )

            # coef -> bf16, PE-transpose, broadcast across partitions
            coef16 = work_pool.tile([P, E], BF16)
            nc.vector.tensor_scalar_mul(coef16[:], coef[:], 1.0)
            pT = psum_pool.tile([E, P], BF16, tag="pT")
            nc.tensor.transpose(pT[:], coef16[:], ident[:])
            coefT = work_pool.tile([E, P], BF16)
            nc.vector.tensor_scalar_mul(coefT[:], pT[:], 1.0)
            coefB = psum_pool.tile([P, E, P], F32, tag="cB")
            for e in range(E):
                nc.tensor.matmul(
                    coefB[:, e, :], lhsT=sel[:, e, :], rhs=coefT[:],
                    start=True, stop=True,
                )
            # uT[j, e, s] = nodeT[j, s] * coef[s, e]
            uT = work_pool.tile([P, E, P], BF16)
            nc.vector.tensor_tensor(
                out=uT[:],
                in0=nodeT[:, bass.ts(t, P)][:, None, :].broadcast_to((P, E, P)),
                in1=coefB[:],
                op=MUL,
            )
            # out[s, i] = sum_e uT_e^T @ W_e, accumulated in PSUM
            po = psum_pool.tile([P, D], F32, tag="po")
            for e in range(E):
                nc.tensor.matmul(
                    po[:], lhsT=uT[:, e, :], rhs=wT[:, e, :],
                    start=(e == 0), stop=(e == E - 1),
                )
            nc.vector.tensor_scalar_mul(acc_all[:, t, :], po[:], 1.0)

        nc.gpsimd.dma_start(out_d, acc_all[:])

    nc.compile()
    return nc


_NC_CACHE = None


def get_nc():
    global _NC_CACHE
    if _NC_CACHE is None:
        _NC_CACHE = build_nc()
    return _NC_CACHE


def make_in_maps(node_state, edge_type_mat, adj_mat, W):
    node_state = np.asarray(node_state, dtype=np.float32)
    edge_type_mat = np.asarray(edge_type_mat, dtype=np.float32)
    adj_mat = np.asarray(adj_mat, dtype=np.float32)
    W = np.asarray(W, dtype=np.float32)

    wT = np.ascontiguousarray(W.transpose(2, 0, 1)).astype(BF16_NP)  # [j, e, i]
    sel = np.zeros((E, E, P), dtype=np.float32)
    for e in range(E):
        sel[e, e, :] = 1.0
    sel = sel.astype(BF16_NP)
    in_maps = []
    for b in range(B):
        edge16 = edge_type_mat[b].astype(BF16_NP)  # [s, o, e]
        edge_t = np.ascontiguousarray(edge16.transpose(0, 2, 1))  # [s, e, o]
        adj16 = adj_mat[b].astype(BF16_NP).reshape(NT, P, N)
        adj_r = np.ascontiguousarray(adj16.transpose(1, 0, 2))  # [p, t, o]
        nodeT = np.ascontiguousarray(node_state[b].T).astype(BF16_NP)  # [j, s]
        in_maps.append(
            {"edge_t": edge_t, "adj_r": adj_r, "nodeT": nodeT, "wT": wT, "sel": sel}
        )
    return in_maps


def kernel(node_state, edge_type_mat, adj_mat, W):
    nc = get_nc()
    in_maps = make_in_maps(node_state, edge_type_mat, adj_mat, W)
    res = run_bass_kernel_spmd(nc, in_maps, list(range(B)))
    # out is [p, t, i] per core -> [s, i] with s = t*P + p
    return np.stack(
        [res.results[b]["out"].transpose(1, 0, 2).reshape(N, D) for b in range(B)],
        axis=0,
    )


# revision 9
# speedup vs baseline: 1.0484x; 1.0195x over previous
"""Trainium2 Bass kernel for nn_MessagePassing (gnn_message_passing).

Math (per batch b):
    coef[s,e] = sum_o adj[s,o] * edge[s,o,e]
    v[s,e,i]  = sum_j W[e,i,j] * node[s,j]
    out[s,i]  = sum_e coef[s,e] * v[s,e,i]

Sharding: data parallel over the batch axis - core b handles batch b.

Host-side staging (per core):
  * edge  -> [s, e, o] bf16: contiguous on-chip reads, half the HBM bytes.
  * adj   -> [p, t, o] bf16, node -> nodeT [j, s] bf16, W -> wT [j, e, i]
    bf16, sel -> e-row selector matrices (constant).
  * out   <- [p, t, i] f32, one contiguous DMA.

Engine split per s-tile (measured HW rates, ns per 1024 elem/partition):
DVE STT fused mult+reduce ~1226, DVE TT bf16-packed-2x ~553 (works with a
stride-0 middle-dim broadcast), DVE TRED ~1086, ACT copy+accum ~1147+278
drain (concurrent with DVE), Pool shares the DVE SBUF port (useless).

  * coef: a_t e's (2 or 3, alternating to balance DVE vs ACT) via fused
    DVE STT; the rest via one 2x DVE TT multiply (adj broadcast over the
    middle dim) + ACT copy+accum reductions.
  * out = sum_e coef[s,e] * (node[s,:] @ W_e) is computed entirely on the
    PE as 8 PSUM-accumulated matmuls with lhsT_e = uT[j,e,s] =
    nodeT[j,s]*coef[s,e].  coef[s,e] must be replicated across the j
    partitions: PE-transpose coef -> coefT[e,s], then 8 selector matmuls
    (lhsT = const one-hot row e) broadcast coefT rows into PSUM, and one
    2x-ineligible DVE TT (PSUM operand) forms uT.
"""

import numpy as np
import ml_dtypes
from contextlib import ExitStack

import concourse.bass as bass
import concourse.bacc as bacc
import concourse.mybir as mybir
import concourse.tile as tile
from concourse.bass_utils import run_bass_kernel_spmd
from concourse.masks import make_identity

B, N, D, E = 8, 1024, 128, 8
P = 128
NT = N // P  # 8 s-tiles per core
CH = 3  # edge chunk split: e<CH arrives first (covers all STT e's)

F32 = mybir.dt.float32
BF16 = mybir.dt.bfloat16
MUL = mybir.AluOpType.mult
ADD = mybir.AluOpType.add
COPY = mybir.ActivationFunctionType.Copy

BF16_NP = ml_dtypes.bfloat16


def build_nc():
    nc = bacc.Bacc("TRN2", target_bir_lowering=False, debug=False, num_devices=B)

    edge_d = nc.dram_tensor("edge_t", [N, E, N], BF16, kind="ExternalInput").ap()
    adj_d = nc.dram_tensor("adj_r", [P, NT, N], BF16, kind="ExternalInput").ap()
    nodeT_d = nc.dram_tensor("nodeT", [D, N], BF16, kind="ExternalInput").ap()
    wT_d = nc.dram_tensor("wT", [D, E, D], BF16, kind="ExternalInput").ap()
    sel_d = nc.dram_tensor("sel", [E, E, P], BF16, kind="ExternalInput").ap()
    out_d = nc.dram_tensor("out", [P, NT, D], F32, kind="ExternalOutput").ap()

    with tile.TileContext(nc) as tc, ExitStack() as ctx:
        const_pool = ctx.enter_context(tc.tile_pool(name="const", bufs=1))
        edge_pool = ctx.enter_context(tc.tile_pool(name="edge", bufs=3))
        prod_pool = ctx.enter_context(tc.tile_pool(name="prod", bufs=2))
        work_pool = ctx.enter_context(tc.tile_pool(name="work", bufs=2))
        psum_pool = ctx.enter_context(tc.tile_pool(name="psum", bufs=2, space="PSUM"))

        adj_all = const_pool.tile([P, NT, N], BF16)
        nodeT = const_pool.tile([P, N], BF16)
        wT = const_pool.tile([P, E, D], BF16)
        sel = const_pool.tile([E, E, P], BF16)
        ident = const_pool.tile([P, P], BF16)
        acc_all = const_pool.tile([P, NT, D], F32)
        scr_v = const_pool.tile([P, N], BF16)  # DVE STT product sink
        scr_a = const_pool.tile([P, N], BF16)  # ACT copy sink

        make_identity(nc, ident[:])

        # Edge stream on the sync queue (tile 0 split so the first STT can
        # start after ~0.8 MiB); everything else on the scalar queue.
        def load_edge(t, split=False):
            et = edge_pool.tile([P, E, N], BF16, tag="edge_t")
            if split:
                nc.sync.dma_start(et[:, 0:CH, :], edge_d[bass.ts(t, P), 0:CH])
                nc.sync.dma_start(et[:, CH:E, :], edge_d[bass.ts(t, P), CH:E])
            else:
                nc.sync.dma_start(et[:], edge_d[bass.ts(t, P)])
            return et

        edge_tiles = {0: load_edge(0, split=True)}
        nc.scalar.dma_start(adj_all[:, 0, :], adj_d[:, 0, :])
        nc.scalar.dma_start(nodeT[:], nodeT_d)
        nc.scalar.dma_start(wT[:], wT_d)
        nc.scalar.dma_start(sel[:], sel_d)
        for t in range(1, NT):
            edge_tiles[t] = load_edge(t)
            nc.scalar.dma_start(adj_all[:, t, :], adj_d[:, t, :])

        for t in range(NT):
            edge_t = edge_tiles[t]
            adj_t = adj_all[:, t, :]
            a = 2 if t % 2 == 0 else 3  # STT e's this tile

            coef = work_pool.tile([P, E], F32)
            # DVE: fused multiply+reduce for e < a
            for e in range(a):
                nc.vector.scalar_tensor_tensor(
                    out=scr_v[:],
                    in0=edge_t[:, e, :],
                    scalar=1.0,
                    in1=adj_t,
                    op0=MUL,
                    op1=MUL,
                    accum_out=coef[:, e : e + 1],
                )
            # DVE: 2x TT multiply for e >= a (adj broadcast over middle dim)
            prod = prod_pool.tile([P, E - 2, N], BF16)
            nc.vector.tensor_tensor(
                out=prod[:, 0 : E - a, :],
                in0=edge_t[:, a:E, :],
                in1=adj_t[:, None, :].broadcast_to((P, E - a, N)),
                op=MUL,
            )
            # ACT: reduce each prod slice -> coef[:, e]
            for e in range(a, E):
                nc.scalar.activation(
                    out=scr_a[:],
                    in_=prod[:, e - a, :],
                    func=COPY,
                    accum_out=coef[:, e : e + 1],
                )

            # coef -> bf16, PE-transpose, broadcast across partitions
            coef16 = work_pool.tile([P, E], BF16)
            nc.vector.tensor_scalar_mul(coef16[:], coef[:], 1.0)
            pT = psum_pool.tile([E, P], BF16, tag="pT")
            nc.tensor.transpose(pT[:], coef16[:], ident[:])
            coefT = work_pool.tile([E, P], BF16)
            nc.vector.tensor_scalar_mul(coefT[:], pT[:], 1.0)
            coefB = psum_pool.tile([P, E, P], F32, tag="cB")
            for e in range(E):
                nc.tensor.matmul(
                    coefB[:, e, :], lhsT=sel[:, e, :], rhs=coefT[:],
                    start=True, stop=True,
                )
            # uT[j, e, s] = nodeT[j, s] * coef[s, e]
            uT = work_pool.tile([P, E, P], BF16)
            nc.vector.tensor_tensor(
                out=uT[:],
                in0=nodeT[:, bass.ts(t, P)][:, None, :].broadcast_to((P, E, P)),
                in1=coefB[:],
                op=MUL,
            )
            # out[s, i] = sum_e uT_e^T @ W_e, accumulated in PSUM
            po = psum_pool.tile([P, D], F32, tag="po")
            for e in range(E):
                nc.tensor.matmul(
                    po[:], lhsT=uT[:, e, :], rhs=wT[:, e, :],
                    start=(e == 0), stop=(e == E - 1),
                )
            nc.vector.tensor_scalar_mul(acc_all[:, t, :], po[:], 1.0)

        nc.gpsimd.dma_start(out_d, acc_all[:])

    nc.compile()
    return nc


_NC_CACHE = None


def get_nc():
    global _NC_CACHE
    if _NC_CACHE is None:
        _NC_CACHE = build_nc()
    return _NC_CACHE


def make_in_maps(node_state, edge_type_mat, adj_mat, W):
    node_state = np.asarray(node_state, dtype=np.float32)
    edge_type_mat = np.asarray(edge_type_mat, dtype=np.float32)
    adj_mat = np.asarray(adj_mat, dtype=np.float32)
    W = np.asarray(W, dtype=np.float32)

    wT = np.ascontiguousarray(W.transpose(2, 0, 1)).astype(BF16_NP)  # [j, e, i]
    sel = np.zeros((E, E, P), dtype=np.float32)
    for e in range(E):
        sel[e, e, :] = 1.0
    sel = sel.astype(BF16_NP)
    in_maps = []
    for b in range(B):
        edge16 = edge_type_mat[b].astype(BF16_NP)  # [s, o, e]
        edge_t = np.ascontiguousarray(edge16.transpose(0, 2, 1))  # [s, e, o]
        adj16 = adj_mat[b].astype(BF16_NP).reshape(NT, P, N)
        adj_r = np.ascontiguousarray(adj16.transpose(1, 0, 2))  # [p, t, o]
        nodeT = np.ascontiguousarray(node_state[b].T).astype(BF16_NP)  # [j, s]
        in_maps.append(
            {"edge_t": edge_t, "adj_r": adj_r, "nodeT": nodeT, "wT": wT, "sel": sel}
        )
    return in_maps


def kernel(node_state, edge_type_mat, adj_mat, W):
    nc = get_nc()
    in_maps = make_in_maps(node_state, edge_type_mat, adj_mat, W)
    res = run_bass_kernel_spmd(nc, in_maps, list(range(B)))
    # out is [p, t, i] per core -> [s, i] with s = t*P + p
    return np.stack(
        [res.results[b]["out"].transpose(1, 0, 2).reshape(N, D) for b in range(B)],
        axis=0,
    )


# revision 12
# speedup vs baseline: 1.2293x; 1.1725x over previous
"""Trainium2 Bass kernel for nn_MessagePassing (gnn_message_passing).

Math (per batch b):
    coef[s,e] = sum_o adj[s,o] * edge[s,o,e]
    v[s,e,i]  = sum_j W[e,i,j] * node[s,j]
    out[s,i]  = sum_e coef[s,e] * v[s,e,i]

Sharding: data parallel over the batch axis - core b handles batch b.

Host-side staging (per core):
  * edge  -> [t, o%128, o//128, e, s%128] bf16: o on SBUF partitions so the
    o-reduction runs on the PE; contiguous 2 MiB DMA per s-tile.
  * adj   -> [o%128, t, o//128, s%128] bf16 (same partition layout).
  * node  -> nodeT [j, s] bf16, W -> wT [j, e, i] bf16.
  * out   <- [p, t, i] f32, one contiguous DMA.

Engine assignment per s-tile (measured HW rates):
  * DVE: 8 2x-TT multiplies prod[o, e, s] = edge * adj (adj broadcast over
    the middle e dim; bf16-packed 2x mode, ~0.54 ns/elem) and one 2x-
    ineligible TT (PSUM operand) forming uT[j,e,s] = nodeT[j,s]*coef[s,e].
  * PE : the ENTIRE reduction: coefRow[1,(e,s)] = sum_o ones^T @ prod
    accumulated over the 8 o-blocks in PSUM, then 2 ones-matmuls that
    broadcast coefRow across partitions, then 8 PSUM-accumulated matmuls
    out[s,i] += uT_e^T @ W_e.
  * ACT: small copies only (coefRow PSUM->SBUF, out PSUM->SBUF).
  * Pool shares the DVE SBUF port - unused.
"""

import numpy as np
import ml_dtypes
from contextlib import ExitStack

import concourse.bass as bass
import concourse.bacc as bacc
import concourse.mybir as mybir
import concourse.tile as tile
from concourse.bass_utils import run_bass_kernel_spmd

B, N, D, E = 8, 1024, 128, 8
P = 128
NT = N // P  # 8 s-tiles per core
OB = N // P  # 8 o-blocks
HF = E * P // 2  # 512: half of the flattened (e, s) row

F32 = mybir.dt.float32
BF16 = mybir.dt.bfloat16
MUL = mybir.AluOpType.mult
COPY = mybir.ActivationFunctionType.Copy

BF16_NP = ml_dtypes.bfloat16


def build_nc():
    nc = bacc.Bacc("TRN2", target_bir_lowering=False, debug=False, num_devices=B)

    edge_d = nc.dram_tensor("edge_t", [NT, P, OB, E, P], BF16, kind="ExternalInput").ap()
    adj_d = nc.dram_tensor("adj_r", [P, NT, OB, P], BF16, kind="ExternalInput").ap()
    nodeT_d = nc.dram_tensor("nodeT", [D, N], BF16, kind="ExternalInput").ap()
    wT_d = nc.dram_tensor("wT", [D, E, D], BF16, kind="ExternalInput").ap()
    out_d = nc.dram_tensor("out", [P, NT, D], F32, kind="ExternalOutput").ap()

    with tile.TileContext(nc) as tc, ExitStack() as ctx:
        const_pool = ctx.enter_context(tc.tile_pool(name="const", bufs=1))
        edge_pool = ctx.enter_context(tc.tile_pool(name="edge", bufs=3))
        prod_pool = ctx.enter_context(tc.tile_pool(name="prod", bufs=2))
        work_pool = ctx.enter_context(tc.tile_pool(name="work", bufs=2))
        ps_row_pool = ctx.enter_context(tc.tile_pool(name="psr", bufs=1, space="PSUM"))
        ps_bc_pool = ctx.enter_context(tc.tile_pool(name="psb", bufs=1, space="PSUM"))
        ps_out_pool = ctx.enter_context(tc.tile_pool(name="pso", bufs=2, space="PSUM"))

        adj_all = const_pool.tile([P, NT, OB, P], BF16)
        nodeT = const_pool.tile([P, N], BF16)
        wT = const_pool.tile([P, E, D], BF16)
        ones_r = const_pool.tile([P, 1], BF16)  # reduce lhsT
        ones_b = const_pool.tile([1, P], BF16)  # broadcast lhsT
        acc_all = const_pool.tile([P, NT, D], F32)

        nc.vector.memset(ones_r[:], 1.0)
        nc.vector.memset(ones_b[:], 1.0)

        # Edge stream on the sync queue (tile 0 split for a fast start);
        # everything else on the scalar queue.
        def load_edge(t, split=False):
            et = edge_pool.tile([P, OB, E, P], BF16, tag="edge_t")
            if split:
                nc.sync.dma_start(et[:, 0:2, :, :], edge_d[t, :, 0:2])
                nc.sync.dma_start(et[:, 2:OB, :, :], edge_d[t, :, 2:OB])
            else:
                nc.sync.dma_start(et[:], edge_d[t])
            return et

        edge_tiles = {0: load_edge(0, split=True)}
        nc.scalar.dma_start(adj_all[:, 0, :, :], adj_d[:, 0, :, :])
        nc.scalar.dma_start(nodeT[:], nodeT_d)
        nc.scalar.dma_start(wT[:], wT_d)
        for t in range(1, NT):
            edge_tiles[t] = load_edge(t)
            nc.scalar.dma_start(adj_all[:, t, :, :], adj_d[:, t, :, :])

        for t in range(NT):
            edge_t = edge_tiles[t]

            # DVE: prod[o, e, s] = edge * adj (2x TT, adj bcast over e) and
            # PE: coefRow[1, (e,s)] += ones^T @ prod, per o-block.
            prod = prod_pool.tile([P, OB, E, P], BF16)
            rowA = ps_row_pool.tile([1, HF], F32, tag="rowA")
            rowB = ps_row_pool.tile([1, HF], F32, tag="rowB")
            rows = [rowA, rowB]
            for ob in range(OB):
                nc.vector.tensor_tensor(
                    out=prod[:, ob, :, :],
                    in0=edge_t[:, ob, :, :],
                    in1=adj_all[:, t, ob, :][:, None, :].broadcast_to((P, E, P)),
                    op=MUL,
                )
                flat = prod[:, ob, :, :].rearrange("p e s -> p (e s)")
                for h in range(2):
                    nc.tensor.matmul(
                        rows[h][:],
                        lhsT=ones_r[:],
                        rhs=flat[:, h * HF : (h + 1) * HF],
                        start=(ob == 0),
                        stop=(ob == OB - 1),
                    )

            # ACT: coefRow PSUM -> SBUF bf16
            coefRow = work_pool.tile([1, E * P], BF16)
            for h in range(2):
                nc.scalar.copy(coefRow[:, h * HF : (h + 1) * HF], rows[h][:])

            # PE: broadcast coefRow across 128 partitions
            bcA = ps_bc_pool.tile([P, HF], F32, tag="bcA")
            bcB = ps_bc_pool.tile([P, HF], F32, tag="bcB")
            coefB = [bcA, bcB]
            for h in range(2):
                nc.tensor.matmul(
                    coefB[h][:],
                    lhsT=ones_b[:],
                    rhs=coefRow[:, h * HF : (h + 1) * HF],
                    start=True,
                    stop=True,
                )

            # DVE: uT[j, e, s] = nodeT[j, s] * coef[s, e]
            uT = work_pool.tile([P, E, P], BF16)
            for h in range(2):
                nc.vector.tensor_tensor(
                    out=uT[:, h * 4 : (h + 1) * 4, :],
                    in0=nodeT[:, bass.ts(t, P)][:, None, :].broadcast_to((P, 4, P)),
                    in1=coefB[h][:].rearrange("p (e s) -> p e s", e=4),
                    op=MUL,
                )

            # PE: out[s, i] = sum_e uT_e^T @ W_e, accumulated in PSUM
            po = ps_out_pool.tile([P, D], F32, tag="po")
            for e in range(E):
                nc.tensor.matmul(
                    po[:], lhsT=uT[:, e, :], rhs=wT[:, e, :],
                    start=(e == 0), stop=(e == E - 1),
                )
            # ACT: out PSUM -> SBUF
            nc.scalar.copy(acc_all[:, t, :], po[:])

        nc.gpsimd.dma_start(out_d, acc_all[:])

    nc.compile()
    return nc


_NC_CACHE = None


def get_nc():
    global _NC_CACHE
    if _NC_CACHE is None:
        _NC_CACHE = build_nc()
    return _NC_CACHE


def make_in_maps(node_state, edge_type_mat, adj_mat, W):
    node_state = np.asarray(node_state, dtype=np.float32)
    edge_type_mat = np.asarray(edge_type_mat, dtype=np.float32)
    adj_mat = np.asarray(adj_mat, dtype=np.float32)
    W = np.asarray(W, dtype=np.float32)

    wT = np.ascontiguousarray(W.transpose(2, 0, 1)).astype(BF16_NP)  # [j, e, i]
    in_maps = []
    for b in range(B):
        edge16 = edge_type_mat[b].astype(BF16_NP)  # [s, o, e]
        # [t, po, ob, e, ps]
        edge_t = np.ascontiguousarray(
            edge16.reshape(NT, P, OB, P, E).transpose(0, 3, 2, 4, 1)
        )
        adj16 = adj_mat[b].astype(BF16_NP).reshape(NT, P, OB, P)
        adj_r = np.ascontiguousarray(adj16.transpose(3, 0, 2, 1))  # [po, t, ob, ps]
        nodeT = np.ascontiguousarray(node_state[b].T).astype(BF16_NP)  # [j, s]
        in_maps.append({"edge_t": edge_t, "adj_r": adj_r, "nodeT": nodeT, "wT": wT})
    return in_maps


def kernel(node_state, edge_type_mat, adj_mat, W):
    nc = get_nc()
    in_maps = make_in_maps(node_state, edge_type_mat, adj_mat, W)
    res = run_bass_kernel_spmd(nc, in_maps, list(range(B)))
    # out is [p, t, i] per core -> [s, i] with s = t*P + p
    return np.stack(
        [res.results[b]["out"].transpose(1, 0, 2).reshape(N, D) for b in range(B)],
        axis=0,
    )


# revision 13
# speedup vs baseline: 1.2847x; 1.0451x over previous
"""Trainium2 Bass kernel for nn_MessagePassing (gnn_message_passing).

Math (per batch b):
    coef[s,e] = sum_o adj[s,o] * edge[s,o,e]
    v[s,e,i]  = sum_j W[e,i,j] * node[s,j]
    out[s,i]  = sum_e coef[s,e] * v[s,e,i]

Sharding: data parallel over the batch axis - core b handles batch b.

Host-side staging (per core):
  * edge  -> [t, o%128, o//128, e, s%128] bf16: o on SBUF partitions so the
    o-reduction runs on the PE; contiguous 2 MiB DMA per s-tile.
  * adj   -> [o%128, t, o//128, s%128] bf16 (same partition layout).
  * node  -> nodeT [j, s] bf16, W -> wT [j, e, i] bf16.
  * out   <- [p, t, i] f32, one contiguous DMA.

Engine assignment per s-tile (measured HW rates):
  * DVE: 8 2x-TT multiplies prod[o, e, s] = edge * adj (adj broadcast over
    the middle e dim; bf16-packed 2x mode, ~0.54 ns/elem) and one 2x-
    ineligible TT (PSUM operand) forming uT[j,e,s] = nodeT[j,s]*coef[s,e].
  * PE : the ENTIRE reduction: coefRow[1,(e,s)] = sum_o ones^T @ prod
    accumulated over the 8 o-blocks in PSUM, then 2 ones-matmuls that
    broadcast coefRow across partitions, then 8 PSUM-accumulated matmuls
    out[s,i] += uT_e^T @ W_e.
  * ACT: small copies only (coefRow PSUM->SBUF, out PSUM->SBUF).
  * Pool shares the DVE SBUF port - unused.
"""

import numpy as np
import ml_dtypes
from contextlib import ExitStack

import concourse.bass as bass
import concourse.bacc as bacc
import concourse.mybir as mybir
import concourse.tile as tile
from concourse.bass_utils import run_bass_kernel_spmd

B, N, D, E = 8, 1024, 128, 8
P = 128
NT = N // P  # 8 s-tiles per core
OB = N // P  # 8 o-blocks
HF = E * P // 2  # 512: half of the flattened (e, s) row

F32 = mybir.dt.float32
BF16 = mybir.dt.bfloat16
MUL = mybir.AluOpType.mult
COPY = mybir.ActivationFunctionType.Copy

BF16_NP = ml_dtypes.bfloat16


def build_nc():
    nc = bacc.Bacc("TRN2", target_bir_lowering=False, debug=False, num_devices=B)

    edge_d = nc.dram_tensor("edge_t", [NT, P, OB, E, P], BF16, kind="ExternalInput").ap()
    adj_d = nc.dram_tensor("adj_r", [P, NT, OB, P], BF16, kind="ExternalInput").ap()
    nodeT_d = nc.dram_tensor("nodeT", [D, N], BF16, kind="ExternalInput").ap()
    wT_d = nc.dram_tensor("wT", [D, E, D], BF16, kind="ExternalInput").ap()
    out_d = nc.dram_tensor("out", [P, NT, D], F32, kind="ExternalOutput").ap()

    with tile.TileContext(nc) as tc, ExitStack() as ctx:
        const_pool = ctx.enter_context(tc.tile_pool(name="const", bufs=1))
        edge_pool = ctx.enter_context(tc.tile_pool(name="edge", bufs=4))
        prod_pool = ctx.enter_context(tc.tile_pool(name="prod", bufs=2))
        work_pool = ctx.enter_context(tc.tile_pool(name="work", bufs=2))
        ps_row_pool = ctx.enter_context(tc.tile_pool(name="psr", bufs=1, space="PSUM"))
        ps_bc_pool = ctx.enter_context(tc.tile_pool(name="psb", bufs=1, space="PSUM"))
        ps_out_pool = ctx.enter_context(tc.tile_pool(name="pso", bufs=2, space="PSUM"))

        adj_all = const_pool.tile([P, NT, OB, P], BF16)
        nodeT = const_pool.tile([P, N], BF16)
        wT = const_pool.tile([P, E, D], BF16)
        ones_r = const_pool.tile([P, 1], BF16)  # reduce lhsT
        ones_b = const_pool.tile([1, P], BF16)  # broadcast lhsT
        acc_all = const_pool.tile([P, NT, D], F32)

        nc.vector.memset(ones_r[:], 1.0)
        nc.vector.memset(ones_b[:], 1.0)

        # Edge stream on the sync queue (tile 0 split for a fast start);
        # everything else on the scalar queue.
        def load_edge(t, split=False):
            et = edge_pool.tile([P, OB, E, P], BF16, tag="edge_t")
            if split:
                nc.sync.dma_start(et[:, 0:1, :, :], edge_d[t, :, 0:1])
                nc.sync.dma_start(et[:, 1:3, :, :], edge_d[t, :, 1:3])
                nc.sync.dma_start(et[:, 3:OB, :, :], edge_d[t, :, 3:OB])
            else:
                nc.sync.dma_start(et[:], edge_d[t])
            return et

        edge_tiles = {0: load_edge(0, split=True)}
        nc.scalar.dma_start(adj_all[:, 0, :, :], adj_d[:, 0, :, :])
        nc.scalar.dma_start(nodeT[:], nodeT_d)
        nc.scalar.dma_start(wT[:], wT_d)
        for t in range(1, NT):
            edge_tiles[t] = load_edge(t)
            nc.scalar.dma_start(adj_all[:, t, :, :], adj_d[:, t, :, :])

        for t in range(NT):
            edge_t = edge_tiles[t]

            # DVE: prod[o, e, s] = edge * adj (2x TT, adj bcast over e) and
            # PE: coefRow[1, (e,s)] += ones^T @ prod, per o-block.
            prod = prod_pool.tile([P, OB, E, P], BF16)
            rowA = ps_row_pool.tile([1, HF], F32, tag="rowA")
            rowB = ps_row_pool.tile([1, HF], F32, tag="rowB")
            rows = [rowA, rowB]
            for ob in range(OB):
                nc.vector.tensor_tensor(
                    out=prod[:, ob, :, :],
                    in0=edge_t[:, ob, :, :],
                    in1=adj_all[:, t, ob, :][:, None, :].broadcast_to((P, E, P)),
                    op=MUL,
                )
                flat = prod[:, ob, :, :].rearrange("p e s -> p (e s)")
                for h in range(2):
                    nc.tensor.matmul(
                        rows[h][:],
                        lhsT=ones_r[:],
                        rhs=flat[:, h * HF : (h + 1) * HF],
                        start=(ob == 0),
                        stop=(ob == OB - 1),
                    )

            # ACT: coefRow PSUM -> SBUF bf16
            coefRow = work_pool.tile([1, E * P], BF16)
            for h in range(2):
                nc.scalar.copy(coefRow[:, h * HF : (h + 1) * HF], rows[h][:])

            # PE: broadcast coefRow across 128 partitions
            bcA = ps_bc_pool.tile([P, HF], F32, tag="bcA")
            bcB = ps_bc_pool.tile([P, HF], F32, tag="bcB")
            coefB = [bcA, bcB]
            for h in range(2):
                nc.tensor.matmul(
                    coefB[h][:],
                    lhsT=ones_b[:],
                    rhs=coefRow[:, h * HF : (h + 1) * HF],
                    start=True,
                    stop=True,
                )

            # DVE: uT[j, e, s] = nodeT[j, s] * coef[s, e]
            uT = work_pool.tile([P, E, P], BF16)
            for h in range(2):
                nc.vector.tensor_tensor(
                    out=uT[:, h * 4 : (h + 1) * 4, :],
                    in0=nodeT[:, bass.ts(t, P)][:, None, :].broadcast_to((P, 4, P)),
                    in1=coefB[h][:].rearrange("p (e s) -> p e s", e=4),
                    op=MUL,
                )

            # PE: out[s, i] = sum_e uT_e^T @ W_e, accumulated in PSUM
            po = ps_out_pool.tile([P, D], F32, tag="po")
            for e in range(E):
                nc.tensor.matmul(
                    po[:], lhsT=uT[:, e, :], rhs=wT[:, e, :],
                    start=(e == 0), stop=(e == E - 1),
                )
            # ACT: out PSUM -> SBUF, then stream out per tile
            nc.scalar.copy(acc_all[:, t, :], po[:])
            nc.gpsimd.dma_start(out_d[:, t, :], acc_all[:, t, :])

    nc.compile()
    return nc


_NC_CACHE = None


def get_nc():
    global _NC_CACHE
    if _NC_CACHE is None:
        _NC_CACHE = build_nc()
    return _NC_CACHE


def make_in_maps(node_state, edge_type_mat, adj_mat, W):
    node_state = np.asarray(node_state, dtype=np.float32)
    edge_type_mat = np.asarray(edge_type_mat, dtype=np.float32)
    adj_mat = np.asarray(adj_mat, dtype=np.float32)
    W = np.asarray(W, dtype=np.float32)

    wT = np.ascontiguousarray(W.transpose(2, 0, 1)).astype(BF16_NP)  # [j, e, i]
    in_maps = []
    for b in range(B):
        edge16 = edge_type_mat[b].astype(BF16_NP)  # [s, o, e]
        # [t, po, ob, e, ps]
        edge_t = np.ascontiguousarray(
            edge16.reshape(NT, P, OB, P, E).transpose(0, 3, 2, 4, 1)
        )
        adj16 = adj_mat[b].astype(BF16_NP).reshape(NT, P, OB, P)
        adj_r = np.ascontiguousarray(adj16.transpose(3, 0, 2, 1))  # [po, t, ob, ps]
        nodeT = np.ascontiguousarray(node_state[b].T).astype(BF16_NP)  # [j, s]
        in_maps.append({"edge_t": edge_t, "adj_r": adj_r, "nodeT": nodeT, "wT": wT})
    return in_maps


def kernel(node_state, edge_type_mat, adj_mat, W):
    nc = get_nc()
    in_maps = make_in_maps(node_state, edge_type_mat, adj_mat, W)
    res = run_bass_kernel_spmd(nc, in_maps, list(range(B)))
    # out is [p, t, i] per core -> [s, i] with s = t*P + p
    return np.stack(
        [res.results[b]["out"].transpose(1, 0, 2).reshape(N, D) for b in range(B)],
        axis=0,
    )
